# revision 19
# baseline (speedup 1.0000x reference)
"""DeepSeek-style attention, tensor-parallel over 8 TRN2 NeuronCores.

Sharding: 16 heads / 8 cores = 2 heads per core. Each core computes its
2 heads' projections, attention, and a partial output projection; the
host sums the 8 partial outputs.

v2 design notes (cost model: matmul = out_free_cols cycles; fp8
DoubleRow = 0.5 cycles/col; ACT/DVE/Pool = free-size elems/cycle):
  - latent transforms are linear -> fused host-side into Wq/Wk
  - QKV projections bf16, full PE rate; V computed pre-transposed
    (x as stationary operand) so no PE transposes are needed
  - scores: lq/lk quantized fp8e4, DoubleRow matmul with a zero
    second k-tile (mega tile = [lq | lk | zeros]) -> 256 cycles per
    (head, tblock, 512 queries)
  - exp on ACT (exact, bf16 out); optional Schraudolph offload of some
    tiles to DVE/Pool (tensor_scalar -> int16 bits of bf16)
  - AV flipped: e[t,s128] is the stationary operand, rhs = vaug[t,65]
    per head ([v | 1]; the ones column accumulates the softmax
    denominator) -> attended^T[s, ch] with den per-partition
  - normalize: reciprocal_approx_fast + per-partition-scale multiply,
    then PE bf16 transpose back to [ch, s] for the output projection
  - output projection bf16; partial outputs written bf16, host sums
"""
import numpy as np
import ml_dtypes

import concourse.mybir as mybir
import concourse.tile as tile
from concourse import bacc
from concourse.bass_utils import run_bass_kernel_spmd

F32 = mybir.dt.float32
BF16 = mybir.dt.bfloat16
FP8 = mybir.dt.float8e4
I16 = mybir.dt.int16
EXP = mybir.ActivationFunctionType.Exp
MUL = mybir.AluOpType.mult
ADD = mybir.AluOpType.add
DR = mybir.MatmulPerfMode.DoubleRow

H, D, HD = 16, 1024, 64
B, S = 2, 2048
BS = B * S          # 4096
KB = D // 128       # 8 k-blocks
NC = 8              # cores
SC = 512            # s-chunk width
NSC = BS // SC      # 8 chunks
TPC = SC // 128     # 4 t-blocks per chunk
NTB = S // 128      # 16 t-blocks per batch
VW = 2 * (HD + 1)   # 130 vaug cols per t-block

# exp engine per (b, sc, tb): 'A' = ACT exact, 'V' = DVE Schraudolph,
# 'P' = Pool Schraudolph.  Tuned against measured rel-err headroom.
EXP_ASSIGN = {}
SCHR_A = 0.125 * 128 * 1.4426950408889634   # scale folded in
SCHR_B = 128 * 127.0 + 0.5 - 5.0

_cache = {}
DEBUG = False


def exp_engine(b, sc, tb):
    return EXP_ASSIGN.get((b, sc, tb), "A")


def build_nc():
    nc = bacc.Bacc("TRN2", target_bir_lowering=False, debug=False)
    xT_d = nc.dram_tensor("xT", [KB, 128, BS], BF16, kind="ExternalInput").ap()
    wq_d = nc.dram_tensor("wq", [128, D], BF16, kind="ExternalInput").ap()
    wk_d = nc.dram_tensor("wk", [128, D], BF16, kind="ExternalInput").ap()
    wv_d = nc.dram_tensor("wv", [128, D], BF16, kind="ExternalInput").ap()
    wo_d = nc.dram_tensor("wo", [128, D], BF16, kind="ExternalInput").ap()
    wf_d = nc.dram_tensor("wf", [128, 4], F32, kind="ExternalInput").ap()
    idb_d = nc.dram_tensor("idb", [128, 128], BF16, kind="ExternalInput").ap()
    out_d = nc.dram_tensor("outT", [KB, 128, BS], BF16, kind="ExternalOutput").ap()
    if DEBUG:
        dbg_mega = nc.dram_tensor("dbg_mega", [128, 3 * BS], FP8, kind="ExternalOutput").ap()
        dbg_vaug = nc.dram_tensor("dbg_vaug", [128, 32 * VW], BF16, kind="ExternalOutput").ap()
        dbg_e = nc.dram_tensor("dbg_e", [128, 2 * SC], BF16, kind="ExternalOutput").ap()
        dbg_att = nc.dram_tensor("dbg_att", [128, 2 * VW], F32, kind="ExternalOutput").ap()
        dbg_attT = nc.dram_tensor("dbg_attT", [128, SC], BF16, kind="ExternalOutput").ap()

    with tile.TileContext(nc) as tc:
        with (
            tc.tile_pool(name="wpool", bufs=1) as wpool,
            tc.tile_pool(name="big", bufs=1) as big,
            tc.tile_pool(name="ep", bufs=3) as epool,
            tc.tile_pool(name="att2", bufs=2) as att2p,
            tc.tile_pool(name="st", bufs=2) as stpool,
            tc.tile_pool(name="scp", bufs=2, space="PSUM") as scpp,
            tc.tile_pool(name="attp", bufs=1, space="PSUM") as attpp,
            tc.tile_pool(name="pp", bufs=2, space="PSUM") as ppp,
        ):
            wq_t = wpool.tile([128, D], BF16, tag="wq")
            wk_t = wpool.tile([128, D], BF16, tag="wk")
            wv_t = wpool.tile([128, D], BF16, tag="wv")
            wo_t = wpool.tile([128, D], BF16, tag="wo")
            wf_t = wpool.tile([128, 4], F32, tag="wf")
            idb_t = wpool.tile([128, 128], BF16, tag="idb")
            nc.sync.dma_start(out=wq_t[:], in_=wq_d)
            blq_s = wf_t[:, 0:1]
            blk_s = wf_t[:, 1:2]

            # mega = [lq | lk | zeros], fp8, k-tile stride BS
            mega = big.tile([128, 3 * BS], FP8, tag="mega")
            mega3 = mega[:].rearrange("p (t n) -> p t n", t=3)
            nc.gpsimd.memset(mega3[:, 2], 0.0)

            vaug = big.tile([128, 32 * VW], BF16, tag="vaug")
            vaug3 = vaug[:].rearrange("p (t c) -> p t c", c=VW)
            vaug4 = vaug[:].rearrange("p (t h c) -> p t h c", h=2, c=HD + 1)
            nc.gpsimd.memset(vaug4[:, :, :, HD:HD + 1], 1.0)

            xt = big.tile([128, NSC * KB * SC], BF16, tag="xt")
            xt4 = xt[:].rearrange("p (c k n) -> p c k n", c=NSC, k=KB)
            nc.sync.dma_start(
                out=xt4[:, 0, 0:4],
                in_=xT_d[0:4, :, 0:SC].rearrange("k p n -> p k n"))
            nc.sync.dma_start(
                out=xt4[:, 0, 4:KB],
                in_=xT_d[4:KB, :, 0:SC].rearrange("k p n -> p k n"))
            for t, d in ((wk_t, wk_d), (wv_t, wv_d), (wo_t, wo_d),
                         (wf_t, wf_d), (idb_t, idb_d)):
                nc.sync.dma_start(out=t[:], in_=d)
            for c in range(1, NSC):
                nc.sync.dma_start(
                    out=xt4[:, c],
                    in_=xT_d[:, :, c * SC:(c + 1) * SC].rearrange("k p n -> p k n"),
                )

            wv3 = wv_t[:].rearrange("p (k n) -> p k n", k=KB)

            # warm the PE p-state during the initial DMA: ~4us of junk
            # matmuls on already-loaded weights (results never read)
            warm = ppp.tile([128, SC], F32, tag="pp")
            for i in range(5):
                nc.tensor.matmul(
                    warm[:], wq_t[:, 0:128], wq_t[:, 0:SC],
                    start=(i == 0), stop=(i == 4))

            # ---------------- emission quanta ----------------
            def chunk_groups(c):
                """Phase 1 for token chunk c as groups of quanta.  Quanta
                within a group share a ppp psum tile and must not interleave
                with other ppp users (the pump enforces this)."""
                col = c * SC
                xv = xt4[:, c]
                state = {}

                def proj_piece(key, w_t, kb0, bias, dst):
                    def f():
                        if kb0 == 0:
                            pt = ppp.tile([128, SC], F32, tag="pp")
                            state[key] = pt
                        p = state[key]
                        for kb in range(kb0, kb0 + 2):
                            nc.tensor.matmul(
                                p[:], w_t[:, kb * 128:(kb + 1) * 128], xv[:, kb],
                                start=(kb == 0), stop=(kb == KB - 1))
                        if kb0 + 2 == KB:
                            nc.vector.tensor_scalar_add(dst, p[:], bias)
                    return f

                def proj_group(key, w_t, bias, dst):
                    return [proj_piece(key, w_t, kb0, bias, dst)
                            for kb0 in range(0, KB, 2)]

                def q_v(i):
                    def f():
                        vp = ppp.tile([128, 128], F32, tag="pp")
                        for kb in range(KB):
                            nc.tensor.matmul(
                                vp[:], xv[:, kb, i * 128:(i + 1) * 128],
                                wv3[:, kb],
                                start=(kb == 0), stop=(kb == KB - 1))
                        tbg = c * TPC + i
                        nc.vector.tensor_copy(
                            out=vaug4[:, tbg, :, 0:HD],
                            in_=vp[:].rearrange("p (h c) -> p h c", h=2))
                    return f

                return [
                    [(q, 430) for q in proj_group("lq", wq_t, blq_s,
                                                  mega3[:, 0, col:col + SC])],
                    [(q, 430) for q in proj_group("lk", wk_t, blk_s,
                                                  mega3[:, 1, col:col + SC])],
                    [(q_v(0), 430)], [(q_v(1), 430)],
                    [(q_v(2), 430)], [(q_v(3), 430)],
                ]

            def emit_scores(b, sc, tb):
                """Scores (fp8 DoubleRow) + exp for one t-block; returns e."""
                scol = b * S + sc * SC
                tcol = b * S + tb * 128
                scp = scpp.tile([128, 2 * SC], F32, tag="scp")
                for h in range(2):
                    nc.tensor.matmul(
                        scp[:, h * SC:(h + 1) * SC],
                        mega3[64 * h:64 * h + 64, 1:3, tcol:tcol + 128],
                        mega3[64 * h:64 * h + 64, 0:2, scol:scol + SC],
                        start=True, stop=True, perf_mode=DR,
                        tile_position=(64 * h, 0))
                e = epool.tile([128, 2 * SC], BF16, tag="e")
                eng = exp_engine(b, sc, tb)
                if eng == "A":
                    nc.scalar.activation(e[:], scp[:], EXP, scale=0.125)
                else:
                    veng = nc.vector if eng == "V" else nc.gpsimd
                    veng.tensor_scalar(
                        out=e[:].bitcast(I16), in0=scp[:],
                        scalar1=SCHR_A, scalar2=SCHR_B, op0=MUL, op1=ADD)
                if DEBUG and b == 0 and sc == 0 and tb == 0:
                    nc.sync.dma_start(out=dbg_e, in_=e[:])
                return e

            def emit_av(b, sc, tb, e, att_ts):
                tbg = b * NTB + tb
                for q in range(4):
                    att = att_ts[q // 2]
                    for h in range(2):
                        nc.tensor.matmul(
                            att[:, q % 2, h * (HD + 1):(h + 1) * (HD + 1)],
                            e[:, h * SC + q * 128:h * SC + (q + 1) * 128],
                            vaug3[:, tbg, h * (HD + 1):(h + 1) * (HD + 1)],
                            start=False, stop=(tb == NTB - 1),
                            skip_group_check=True)

            def emit_finish_part1(b, sc, att_ts):
                """Normalize: recip + per-partition scale into attTt.
                Emitted immediately after AV(sc, 15) so the att psum tiles
                free up for the next s-chunk."""
                if DEBUG and b == 0 and sc == 0:
                    dbg_att_s = att2p.tile([128, 2 * VW], F32, tag="dbga")
                    nc.vector.tensor_copy(
                        out=dbg_att_s[:].rearrange("p (s c) -> p s c", c=VW),
                        in_=att_ts[0][:])
                    nc.sync.dma_start(out=dbg_att, in_=dbg_att_s[:])
                rec = att2p.tile([128, 8], F32, tag="rec")
                attTt = att2p.tile([128, 4 * 128], BF16, tag="attTt")
                for t_i in range(2):
                    a4 = att_ts[t_i][:].rearrange("p s (h c) -> p s h c", c=HD + 1)
                    nc.vector.reciprocal_approx_fast(
                        out=rec[:, 4 * t_i:4 * t_i + 4].rearrange(
                            "p (s h) -> p s h", s=2),
                        in_=a4[:, :, :, HD:HD + 1].rearrange("p s h o -> p s (h o)"))
                for q in range(4):
                    att = att_ts[q // 2]
                    a3 = att[:, q % 2].rearrange("p (h c) -> p h c", c=HD + 1)
                    last = (b == B - 1 and sc == NSC // B - 1)
                    for h in range(2):
                        dst = attTt[:, q * 128 + h * HD:q * 128 + (h + 1) * HD]
                        if last:
                            nc.scalar.mul(dst, a3[:, h, 0:HD],
                                          rec[:, 2 * q + h:2 * q + h + 1])
                        else:
                            nc.vector.tensor_scalar_mul(
                                dst, a3[:, h, 0:HD],
                                rec[:, 2 * q + h:2 * q + h + 1])
                return attTt

            def finish_part2_quanta(b, sc, attTt):
                """Transpose attended back to [ch, s], out-proj, stage, DMA."""
                scol = b * S + sc * SC
                attT = att2p.tile([128, SC], BF16, tag="attT")

                def q_tr(qr):
                    def f():
                        for q in qr:
                            tp = ppp.tile([128, 128], BF16, tag="pp")
                            nc.tensor.transpose(
                                tp[:], attTt[:, q * 128:(q + 1) * 128], idb_t[:])
                            nc.vector.tensor_copy(
                                out=attT[:, q * 128:(q + 1) * 128], in_=tp[:])
                        if DEBUG and b == 0 and sc == 0 and qr[-1] == 3:
                            nc.sync.dma_start(out=dbg_attT, in_=attT[:])
                    return f

                def q_oproj(pair):
                    def f():
                        stage = stpool.tile([128, 2 * SC], BF16, tag="stage")
                        for jj in range(2):
                            j = pair * 2 + jj
                            pop = ppp.tile([128, SC], F32, tag="pp")
                            nc.tensor.matmul(
                                pop[:], wo_t[:, j * 128:(j + 1) * 128], attT[:],
                                start=True, stop=True)
                            nc.vector.tensor_copy(
                                out=stage[:, jj * SC:(jj + 1) * SC], in_=pop[:])
                        nc.sync.dma_start(
                            out=out_d[pair * 2:pair * 2 + 2, :, scol:scol + SC]
                                .rearrange("k p n -> p k n"),
                            in_=stage[:].rearrange("p (k n) -> p k n", k=2),
                        )
                    return f

                last = (b == B - 1 and sc == NSC // B - 1)
                if not last:
                    return [(q_tr((0, 1)), 110), (q_tr((2, 3)), 110),
                            (q_oproj(0), 430), (q_oproj(1), 430),
                            (q_oproj(2), 430), (q_oproj(3), 430)]

                # last s-chunk: pipeline per query-block so the tail chain
                # is short; stage is one tile, DMAs fire per j-pair at end
                def q_lastq(q):
                    def f():
                        stage = last_stage
                        tp = ppp.tile([128, 128], BF16, tag="pp")
                        nc.tensor.transpose(
                            tp[:], attTt[:, q * 128:(q + 1) * 128], idb_t[:])
                        nc.vector.tensor_copy(
                            out=attT[:, q * 128:(q + 1) * 128], in_=tp[:])
                        for j in range(KB):
                            pop = ppp.tile([128, 128], F32, tag="pp")
                            nc.tensor.matmul(
                                pop[:], wo_t[:, j * 128:(j + 1) * 128],
                                attT[:, q * 128:(q + 1) * 128],
                                start=True, stop=True)
                            eng = nc.scalar if j % 2 == 0 else nc.vector
                            dst = stage[:, j, q * 128:(q + 1) * 128]
                            if j % 2 == 0:
                                nc.scalar.copy(out=dst, in_=pop[:])
                            else:
                                nc.vector.tensor_copy(out=dst, in_=pop[:])
                    return f

                def q_lastdma(pair):
                    def f():
                        nc.sync.dma_start(
                            out=out_d[pair * 2:pair * 2 + 2, :, scol:scol + SC]
                                .rearrange("k p n -> p k n"),
                            in_=last_stage[:, pair * 2:pair * 2 + 2, :],
                        )
                    return f

                last_stage = stpool.tile([128, KB, SC], BF16, tag="lstage")
                return ([(q_lastq(q), 560) for q in range(4)] +
                        [(q_lastdma(p), 10) for p in range(4)])

            # ---------------- software-pipelined emission ----------------
            # Slot order is [scores(i), pump, AV(i-1)]: pumped phase-1 /
            # finish work runs while exp(i-1) is in flight on ACT, and the
            # next scores follow the AV drain immediately.
            pending = []        # (chunk_idx, quantum, last_in_group)
            finish_q = []
            in_group = [False]

            def add_chunk(c, groups):
                for g in groups:
                    for i, (q, cost) in enumerate(g):
                        pending.append((c, q, i == len(g) - 1, cost))

            g0 = chunk_groups(0)
            for q, _ in g0[0] + g0[1]:  # chunk0 lq + lk: critical path
                q()
            add_chunk(0, g0[2:])
            for c in range(1, NSC):
                add_chunk(c, chunk_groups(c))

            def pump(budget, due):
                # Drain quanta up to ~budget ns of PE work; only start chunk
                # work that is due within the next 2 chunks.
                while budget > 0:
                    if in_group[0] and pending:
                        _, q, last, cost = pending.pop(0)
                        q()
                        in_group[0] = not last
                        budget -= cost
                    elif finish_q:
                        q, cost = finish_q.pop(0)
                        q()
                        budget -= cost
                    elif pending and pending[0][0] <= due + 3:
                        _, q, last, cost = pending.pop(0)
                        q()
                        in_group[0] = not last
                        budget -= cost
                    else:
                        break

            def pump_until_chunk(cidx):
                while pending and (in_group[0] or pending[0][0] <= cidx):
                    _, q, last, cost = pending.pop(0)
                    q()
                    in_group[0] = not last

            slots = [(b, sc, tb)
                     for b in range(B)
                     for sc in range(NSC // B)
                     for tb in range(NTB)]
            prev = None          # (b, sc, tb, e)
            cur_att = None
            prev_sc = None

            def av_for(slot_state):
                nonlocal cur_att, prev_sc
                b, sc, tb, e = slot_state
                if tb == 0:
                    if cur_att is not None:
                        attTt = emit_finish_part1(*prev_sc, cur_att)
                        finish_q.extend(finish_part2_quanta(*prev_sc, attTt))
                    att_a = attpp.tile([128, 2, VW], F32, tag="att_a")
                    att_b = attpp.tile([128, 2, VW], F32, tag="att_b")
                    cur_att = (att_a, att_b)
                    prev_sc = (b, sc)
                    nc.vector.memset(att_a[:], 0.0)
                    nc.vector.memset(att_b[:], 0.0)
                emit_av(b, sc, tb, e, cur_att)

            for b, sc, tb in slots:
                due = b * (NSC // B) + max(sc, tb // TPC)
                pump_until_chunk(due)
                e = emit_scores(b, sc, tb)
                pump(550, due)
                if prev is not None:
                    av_for(prev)
                prev = (b, sc, tb, e)
            av_for(prev)
            attTt = emit_finish_part1(*prev_sc, cur_att)
            finish_q.extend(finish_part2_quanta(*prev_sc, attTt))
            while pending or finish_q:
                pump(10000, NSC)

    nc.compile()
    return nc


def _prep_inputs(x, Wq, Wk, Wv, Wo, Wlq, blq, Wlk, blk):
    bf = ml_dtypes.bfloat16
    x = np.asarray(x, np.float64)
    xT = np.ascontiguousarray(x.reshape(BS, D).T).reshape(KB, 128, BS).astype(bf)

    Wq = np.asarray(Wq, np.float64)
    Wk = np.asarray(Wk, np.float64)
    Wv = np.asarray(Wv, np.float64)
    Wo = np.asarray(Wo, np.float64)
    Wlq = np.asarray(Wlq, np.float64)
    Wlk = np.asarray(Wlk, np.float64)
    blq64 = np.asarray(blq, np.float64)
    blk64 = np.asarray(blk, np.float64)

    def sbl(w_c):  # [128 rows, D] -> [128, D] kb-major lhsT layout
        return np.ascontiguousarray(
            w_c.T.reshape(KB, 128, 128).transpose(1, 0, 2).reshape(128, D))

    wf = np.zeros((128, 4), np.float32)
    wf[0:HD, 0] = blq64
    wf[HD:128, 0] = blq64
    wf[0:HD, 1] = blk64
    wf[HD:128, 1] = blk64
    idb = np.eye(128).astype(bf)

    in_maps = []
    for c in range(NC):
        r = slice(c * 128, (c + 1) * 128)
        wq_f = np.empty((128, D), np.float64)
        wk_f = np.empty((128, D), np.float64)
        wq_c, wk_c = Wq[r, :], Wk[r, :]
        wq_f[0:HD] = Wlq @ wq_c[0:HD]
        wq_f[HD:128] = Wlq @ wq_c[HD:128]
        wk_f[0:HD] = Wlk @ wk_c[0:HD]
        wk_f[HD:128] = Wlk @ wk_c[HD:128]
        in_maps.append({
            "xT": xT,
            "wq": sbl(wq_f).astype(bf),
            "wk": sbl(wk_f).astype(bf),
            "wv": sbl(Wv[r, :]).astype(bf),
            "wo": np.ascontiguousarray(Wo[:, r].T).astype(bf),
            "wf": wf,
            "idb": idb,
        })
    return in_maps


def kernel(x, Wq, Wk, Wv, Wo, Wlq, blq, Wlk, blk):
    if "nc" not in _cache:
        _cache["nc"] = build_nc()
    nc = _cache["nc"]
    in_maps = _prep_inputs(x, Wq, Wk, Wv, Wo, Wlq, blq, Wlk, blk)
    res = run_bass_kernel_spmd(nc, in_maps, core_ids=list(range(NC)))
    acc = np.zeros((KB, 128, BS), np.float32)
    for c in range(NC):
        acc += res.results[c]["outT"].astype(np.float32)
    out = acc.reshape(D, BS).T.reshape(B, S, D).astype(np.float32)
    return out


# revision 20
# speedup vs baseline: 1.0437x; 1.0437x over previous
"""DeepSeek-style attention, tensor-parallel over 8 TRN2 NeuronCores.

Sharding: 16 heads / 8 cores = 2 heads per core. Each core computes its
2 heads' projections, attention, and a partial output projection; the
host sums the 8 partial outputs.

v2 design notes (cost model: matmul = out_free_cols cycles; fp8
DoubleRow = 0.5 cycles/col; ACT/DVE/Pool = free-size elems/cycle):
  - latent transforms are linear -> fused host-side into Wq/Wk
  - QKV projections bf16, full PE rate; V computed pre-transposed
    (x as stationary operand) so no PE transposes are needed
  - scores: lq/lk quantized fp8e4, DoubleRow matmul with a zero
    second k-tile (mega tile = [lq | lk | zeros]) -> 256 cycles per
    (head, tblock, 512 queries)
  - exp on ACT (exact, bf16 out); optional Schraudolph offload of some
    tiles to DVE/Pool (tensor_scalar -> int16 bits of bf16)
  - AV flipped: e[t,s128] is the stationary operand, rhs = vaug[t,65]
    per head ([v | 1]; the ones column accumulates the softmax
    denominator) -> attended^T[s, ch] with den per-partition
  - normalize: reciprocal_approx_fast + per-partition-scale multiply,
    then PE bf16 transpose back to [ch, s] for the output projection
  - output projection bf16; partial outputs written bf16, host sums
"""
import numpy as np
import ml_dtypes

import concourse.mybir as mybir
import concourse.tile as tile
from concourse import bacc
from concourse.bass_utils import run_bass_kernel_spmd

F32 = mybir.dt.float32
BF16 = mybir.dt.bfloat16
FP8 = mybir.dt.float8e4
I16 = mybir.dt.int16
EXP = mybir.ActivationFunctionType.Exp
MUL = mybir.AluOpType.mult
ADD = mybir.AluOpType.add
DR = mybir.MatmulPerfMode.DoubleRow

H, D, HD = 16, 1024, 64
B, S = 2, 2048
BS = B * S          # 4096
KB = D // 128       # 8 k-blocks
NC = 8              # cores
SC = 512            # s-chunk width
NSC = BS // SC      # 8 chunks
TPC = SC // 128     # 4 t-blocks per chunk
NTB = S // 128      # 16 t-blocks per batch
VW = 2 * (HD + 1)   # 130 vaug cols per t-block

# exp engine per (b, sc, tb): 'A' = ACT exact, 'V' = DVE Schraudolph,
# 'P' = Pool Schraudolph.  Tuned against measured rel-err headroom.
EXP_ASSIGN = {}
SCHR_A = 0.125 * 128 * 1.4426950408889634   # scale folded in
SCHR_B = 128 * 127.0 + 0.5 - 5.0

_cache = {}
DEBUG = False


def exp_engine(b, sc, tb):
    return EXP_ASSIGN.get((b, sc, tb), "A")


def build_nc():
    nc = bacc.Bacc("TRN2", target_bir_lowering=False, debug=False)
    xh_d = nc.dram_tensor("xh", [KB, 128, BS], FP8, kind="ExternalInput").ap()
    xl_d = nc.dram_tensor("xl", [KB, 128, BS], FP8, kind="ExternalInput").ap()
    xs_d = nc.dram_tensor("xs", [KB, 128, BS], FP8, kind="ExternalInput").ap()
    wq_d = nc.dram_tensor("wq", [128, 3 * D], FP8, kind="ExternalInput").ap()
    wk_d = nc.dram_tensor("wk", [128, 3 * D], FP8, kind="ExternalInput").ap()
    wv_d = nc.dram_tensor("wv", [128, 3 * D], FP8, kind="ExternalInput").ap()
    wo_d = nc.dram_tensor("wo", [128, D], BF16, kind="ExternalInput").ap()
    wf_d = nc.dram_tensor("wf", [128, 4], F32, kind="ExternalInput").ap()
    idb_d = nc.dram_tensor("idb", [128, 128], BF16, kind="ExternalInput").ap()
    out_d = nc.dram_tensor("outT", [KB, 128, BS], BF16, kind="ExternalOutput").ap()
    if DEBUG:
        dbg_mega = nc.dram_tensor("dbg_mega", [128, 3 * BS], FP8, kind="ExternalOutput").ap()
        dbg_vaug = nc.dram_tensor("dbg_vaug", [128, 32 * VW], BF16, kind="ExternalOutput").ap()
        dbg_e = nc.dram_tensor("dbg_e", [128, 2 * SC], BF16, kind="ExternalOutput").ap()
        dbg_att = nc.dram_tensor("dbg_att", [128, 2 * VW], F32, kind="ExternalOutput").ap()
        dbg_attT = nc.dram_tensor("dbg_attT", [128, SC], BF16, kind="ExternalOutput").ap()

    with tile.TileContext(nc) as tc:
        with (
            tc.tile_pool(name="wpool", bufs=1) as wpool,
            tc.tile_pool(name="big", bufs=1) as big,
            tc.tile_pool(name="ep", bufs=3) as epool,
            tc.tile_pool(name="att2", bufs=2) as att2p,
            tc.tile_pool(name="st", bufs=2) as stpool,
            tc.tile_pool(name="scp", bufs=2, space="PSUM") as scpp,
            tc.tile_pool(name="attp", bufs=1, space="PSUM") as attpp,
            tc.tile_pool(name="pp", bufs=2, space="PSUM") as ppp,
        ):
            wq_t = wpool.tile([128, 3 * D], FP8, tag="wq")
            wk_t = wpool.tile([128, 3 * D], FP8, tag="wk")
            wv_t = wpool.tile([128, 3 * D], FP8, tag="wv")
            wo_t = wpool.tile([128, D], BF16, tag="wo")
            wf_t = wpool.tile([128, 4], F32, tag="wf")
            idb_t = wpool.tile([128, 128], BF16, tag="idb")
            nc.sync.dma_start(out=wq_t[:], in_=wq_d)
            # per-weight views [128, term, kb, 128]
            wq4 = wq_t[:].rearrange("p (t k n) -> p t k n", t=3, k=KB)
            wk4 = wk_t[:].rearrange("p (t k n) -> p t k n", t=3, k=KB)
            wv4 = wv_t[:].rearrange("p (t k n) -> p t k n", t=3, k=KB)
            blq_s = wf_t[:, 0:1]
            blk_s = wf_t[:, 1:2]

            # mega = [lq | lk | zeros], fp8, k-tile stride BS
            mega = big.tile([128, 3 * BS], FP8, tag="mega")
            mega3 = mega[:].rearrange("p (t n) -> p t n", t=3)
            nc.gpsimd.memset(mega3[:, 2], 0.0)

            vaug = big.tile([128, 32 * VW], BF16, tag="vaug")
            vaug3 = vaug[:].rearrange("p (t c) -> p t c", c=VW)
            vaug4 = vaug[:].rearrange("p (t h c) -> p t h c", h=2, c=HD + 1)
            nc.gpsimd.memset(vaug4[:, :, :, HD:HD + 1], 1.0)

            xh_t = big.tile([128, NSC * KB * SC], FP8, tag="xh")
            xl_t = big.tile([128, NSC * KB * SC], FP8, tag="xl")
            xs_t = big.tile([128, NSC * KB * SC], FP8, tag="xs")
            xh4 = xh_t[:].rearrange("p (c k n) -> p c k n", c=NSC, k=KB)
            xl4 = xl_t[:].rearrange("p (c k n) -> p c k n", c=NSC, k=KB)
            xs4 = xs_t[:].rearrange("p (c k n) -> p c k n", c=NSC, k=KB)
            for t4, d in ((xh4, xh_d), (xl4, xl_d), (xs4, xs_d)):
                nc.sync.dma_start(
                    out=t4[:, 0],
                    in_=d[:, :, 0:SC].rearrange("k p n -> p k n"))
            for t, d in ((wk_t, wk_d), (wv_t, wv_d), (wo_t, wo_d),
                         (wf_t, wf_d), (idb_t, idb_d)):
                nc.sync.dma_start(out=t[:], in_=d)
            for c in range(1, NSC):
                for t4, d in ((xh4, xh_d), (xl4, xl_d), (xs4, xs_d)):
                    nc.sync.dma_start(
                        out=t4[:, c],
                        in_=d[:, :, c * SC:(c + 1) * SC].rearrange("k p n -> p k n"),
                    )

            # warm the PE p-state during the initial DMA: ~4us of junk
            # matmuls on already-loaded weights (results never read)
            warm = ppp.tile([128, SC], F32, tag="pp")
            for i in range(5):
                nc.tensor.matmul(
                    warm[:], wq_t[:, 0:128], wq_t[:, 0:SC],
                    start=(i == 0), stop=(i == 4))
            X3 = (xh4, xl4, xs4)  # matched term scales: 16*(x@W)

            # ---------------- emission quanta ----------------
            def chunk_groups(c):
                """Phase 1 for token chunk c as groups of quanta.  Quanta
                within a group share a ppp psum tile and must not interleave
                with other ppp users (the pump enforces this)."""
                col = c * SC
                state = {}

                def proj_piece(key, w4, term, kb0, bias, dst):
                    # DoubleRow over kb pairs; 3 scale-matched fp8 terms
                    # accumulate 16*(x@W) into one psum
                    def f():
                        if term == 0 and kb0 == 0:
                            pt = ppp.tile([128, SC], F32, tag="pp")
                            state[key] = pt
                        p = state[key]
                        xv = X3[term][:, c]
                        for kb in range(kb0, kb0 + 4, 2):
                            nc.tensor.matmul(
                                p[:],
                                w4[:, term, kb:kb + 2, :],
                                xv[:, kb:kb + 2, :],
                                start=(term == 0 and kb == 0),
                                stop=(term == 2 and kb == KB - 2),
                                perf_mode=DR)
                        if term == 2 and kb0 + 4 == KB:
                            nc.vector.tensor_scalar(
                                out=dst, in0=p[:], scalar1=0.0625,
                                scalar2=bias, op0=MUL, op1=ADD)
                    return f

                def proj_group(key, w4, bias, dst):
                    return [proj_piece(key, w4, term, kb0, bias, dst)
                            for term in range(3) for kb0 in (0, 4)]

                def q_v(i):
                    def f():
                        vp = ppp.tile([128, 128], F32, tag="pp")
                        for term in range(3):
                            xv = X3[term][:, c]
                            for kb in range(0, KB, 2):
                                nc.tensor.matmul(
                                    vp[:],
                                    xv[:, kb:kb + 2, i * 128:(i + 1) * 128],
                                    wv4[:, term, kb:kb + 2, :],
                                    start=(term == 0 and kb == 0),
                                    stop=(term == 2 and kb == KB - 2),
                                    perf_mode=DR)
                        tbg = c * TPC + i
                        nc.vector.tensor_scalar_mul(
                            vaug4[:, tbg, :, 0:HD],
                            vp[:].rearrange("p (h c) -> p h c", h=2),
                            0.0625)
                    return f

                return [
                    [(q, 240) for q in proj_group("lq", wq4, blq_s,
                                                  mega3[:, 0, col:col + SC])],
                    [(q, 240) for q in proj_group("lk", wk4, blk_s,
                                                  mega3[:, 1, col:col + SC])],
                    [(q_v(0), 350)], [(q_v(1), 350)],
                    [(q_v(2), 350)], [(q_v(3), 350)],
                ]

            def emit_scores(b, sc, tb):
                """Scores (fp8 DoubleRow) + exp for one t-block; returns e."""
                scol = b * S + sc * SC
                tcol = b * S + tb * 128
                scp = scpp.tile([128, 2 * SC], F32, tag="scp")
                for h in range(2):
                    nc.tensor.matmul(
                        scp[:, h * SC:(h + 1) * SC],
                        mega3[64 * h:64 * h + 64, 1:3, tcol:tcol + 128],
                        mega3[64 * h:64 * h + 64, 0:2, scol:scol + SC],
                        start=True, stop=True, perf_mode=DR,
                        tile_position=(64 * h, 0))
                e = epool.tile([128, 2 * SC], BF16, tag="e")
                eng = exp_engine(b, sc, tb)
                if eng == "A":
                    nc.scalar.activation(e[:], scp[:], EXP, scale=0.125)
                else:
                    veng = nc.vector if eng == "V" else nc.gpsimd
                    veng.tensor_scalar(
                        out=e[:].bitcast(I16), in0=scp[:],
                        scalar1=SCHR_A, scalar2=SCHR_B, op0=MUL, op1=ADD)
                if DEBUG and b == 0 and sc == 0 and tb == 0:
                    nc.sync.dma_start(out=dbg_e, in_=e[:])
                return e

            def emit_av(b, sc, tb, e, att_ts):
                tbg = b * NTB + tb
                for q in range(4):
                    att = att_ts[q // 2]
                    for h in range(2):
                        nc.tensor.matmul(
                            att[:, q % 2, h * (HD + 1):(h + 1) * (HD + 1)],
                            e[:, h * SC + q * 128:h * SC + (q + 1) * 128],
                            vaug3[:, tbg, h * (HD + 1):(h + 1) * (HD + 1)],
                            start=False, stop=(tb == NTB - 1),
                            skip_group_check=True)

            def emit_finish_part1(b, sc, att_ts):
                """Normalize: recip + per-partition scale into attTt.
                Emitted immediately after AV(sc, 15) so the att psum tiles
                free up for the next s-chunk."""
                if DEBUG and b == 0 and sc == 0:
                    dbg_att_s = att2p.tile([128, 2 * VW], F32, tag="dbga")
                    nc.vector.tensor_copy(
                        out=dbg_att_s[:].rearrange("p (s c) -> p s c", c=VW),
                        in_=att_ts[0][:])
                    nc.sync.dma_start(out=dbg_att, in_=dbg_att_s[:])
                rec = att2p.tile([128, 8], F32, tag="rec")
                attTt = att2p.tile([128, 4 * 128], BF16, tag="attTt")
                for t_i in range(2):
                    a4 = att_ts[t_i][:].rearrange("p s (h c) -> p s h c", c=HD + 1)
                    nc.vector.reciprocal_approx_fast(
                        out=rec[:, 4 * t_i:4 * t_i + 4].rearrange(
                            "p (s h) -> p s h", s=2),
                        in_=a4[:, :, :, HD:HD + 1].rearrange("p s h o -> p s (h o)"))
                for q in range(4):
                    att = att_ts[q // 2]
                    a3 = att[:, q % 2].rearrange("p (h c) -> p h c", c=HD + 1)
                    last = (b == B - 1 and sc == NSC // B - 1)
                    for h in range(2):
                        dst = attTt[:, q * 128 + h * HD:q * 128 + (h + 1) * HD]
                        if last:
                            nc.scalar.mul(dst, a3[:, h, 0:HD],
                                          rec[:, 2 * q + h:2 * q + h + 1])
                        else:
                            nc.vector.tensor_scalar_mul(
                                dst, a3[:, h, 0:HD],
                                rec[:, 2 * q + h:2 * q + h + 1])
                return attTt

            def finish_part2_quanta(b, sc, attTt):
                """Transpose attended back to [ch, s], out-proj, stage, DMA."""
                scol = b * S + sc * SC
                attT = att2p.tile([128, SC], BF16, tag="attT")

                def q_tr(qr):
                    def f():
                        for q in qr:
                            tp = ppp.tile([128, 128], BF16, tag="pp")
                            nc.tensor.transpose(
                                tp[:], attTt[:, q * 128:(q + 1) * 128], idb_t[:])
                            nc.vector.tensor_copy(
                                out=attT[:, q * 128:(q + 1) * 128], in_=tp[:])
                        if DEBUG and b == 0 and sc == 0 and qr[-1] == 3:
                            nc.sync.dma_start(out=dbg_attT, in_=attT[:])
                    return f

                def q_oproj(pair):
                    def f():
                        stage = stpool.tile([128, 2 * SC], BF16, tag="stage")
                        for jj in range(2):
                            j = pair * 2 + jj
                            pop = ppp.tile([128, SC], F32, tag="pp")
                            nc.tensor.matmul(
                                pop[:], wo_t[:, j * 128:(j + 1) * 128], attT[:],
                                start=True, stop=True)
                            nc.vector.tensor_copy(
                                out=stage[:, jj * SC:(jj + 1) * SC], in_=pop[:])
                        nc.sync.dma_start(
                            out=out_d[pair * 2:pair * 2 + 2, :, scol:scol + SC]
                                .rearrange("k p n -> p k n"),
                            in_=stage[:].rearrange("p (k n) -> p k n", k=2),
                        )
                    return f

                last = (b == B - 1 and sc == NSC // B - 1)
                if not last:
                    return [(q_tr((0, 1)), 110), (q_tr((2, 3)), 110),
                            (q_oproj(0), 430), (q_oproj(1), 430),
                            (q_oproj(2), 430), (q_oproj(3), 430)]

                # last s-chunk: pipeline per query-block so the tail chain
                # is short; stage is one tile, DMAs fire per j-pair at end
                def q_lastq(q):
                    def f():
                        stage = last_stage
                        tp = ppp.tile([128, 128], BF16, tag="pp")
                        nc.tensor.transpose(
                            tp[:], attTt[:, q * 128:(q + 1) * 128], idb_t[:])
                        nc.vector.tensor_copy(
                            out=attT[:, q * 128:(q + 1) * 128], in_=tp[:])
                        for j in range(KB):
                            pop = ppp.tile([128, 128], F32, tag="pp")
                            nc.tensor.matmul(
                                pop[:], wo_t[:, j * 128:(j + 1) * 128],
                                attT[:, q * 128:(q + 1) * 128],
                                start=True, stop=True)
                            eng = nc.scalar if j % 2 == 0 else nc.vector
                            dst = stage[:, j, q * 128:(q + 1) * 128]
                            if j % 2 == 0:
                                nc.scalar.copy(out=dst, in_=pop[:])
                            else:
                                nc.vector.tensor_copy(out=dst, in_=pop[:])
                    return f

                def q_lastdma(pair):
                    def f():
                        nc.sync.dma_start(
                            out=out_d[pair * 2:pair * 2 + 2, :, scol:scol + SC]
                                .rearrange("k p n -> p k n"),
                            in_=last_stage[:, pair * 2:pair * 2 + 2, :],
                        )
                    return f

                last_stage = stpool.tile([128, KB, SC], BF16, tag="lstage")
                return ([(q_lastq(q), 560) for q in range(4)] +
                        [(q_lastdma(p), 10) for p in range(4)])

            # ---------------- software-pipelined emission ----------------
            # Slot order is [scores(i), pump, AV(i-1)]: pumped phase-1 /
            # finish work runs while exp(i-1) is in flight on ACT, and the
            # next scores follow the AV drain immediately.
            pending = []        # (chunk_idx, quantum, last_in_group)
            finish_q = []
            in_group = [False]

            def add_chunk(c, groups):
                for g in groups:
                    for i, (q, cost) in enumerate(g):
                        pending.append((c, q, i == len(g) - 1, cost))

            g0 = chunk_groups(0)
            for q, _ in g0[0] + g0[1]:  # chunk0 lq + lk: critical path
                q()
            add_chunk(0, g0[2:])
            for c in range(1, NSC):
                add_chunk(c, chunk_groups(c))

            def pump(budget, due):
                # Drain quanta up to ~budget ns of PE work; only start chunk
                # work that is due within the next 2 chunks.
                while budget > 0:
                    if in_group[0] and pending:
                        _, q, last, cost = pending.pop(0)
                        q()
                        in_group[0] = not last
                        budget -= cost
                    elif finish_q:
                        q, cost = finish_q.pop(0)
                        q()
                        budget -= cost
                    elif pending and pending[0][0] <= due + 3:
                        _, q, last, cost = pending.pop(0)
                        q()
                        in_group[0] = not last
                        budget -= cost
                    else:
                        break

            def pump_until_chunk(cidx):
                while pending and (in_group[0] or pending[0][0] <= cidx):
                    _, q, last, cost = pending.pop(0)
                    q()
                    in_group[0] = not last

            slots = [(b, sc, tb)
                     for b in range(B)
                     for sc in range(NSC // B)
                     for tb in range(NTB)]
            prev = None          # (b, sc, tb, e)
            cur_att = None
            prev_sc = None

            def av_for(slot_state):
                nonlocal cur_att, prev_sc
                b, sc, tb, e = slot_state
                if tb == 0:
                    if cur_att is not None:
                        attTt = emit_finish_part1(*prev_sc, cur_att)
                        finish_q.extend(finish_part2_quanta(*prev_sc, attTt))
                    att_a = attpp.tile([128, 2, VW], F32, tag="att_a")
                    att_b = attpp.tile([128, 2, VW], F32, tag="att_b")
                    cur_att = (att_a, att_b)
                    prev_sc = (b, sc)
                    nc.vector.memset(att_a[:], 0.0)
                    nc.vector.memset(att_b[:], 0.0)
                emit_av(b, sc, tb, e, cur_att)

            for b, sc, tb in slots:
                due = b * (NSC // B) + max(sc, tb // TPC)
                pump_until_chunk(due)
                e = emit_scores(b, sc, tb)
                pump(550, due)
                if prev is not None:
                    av_for(prev)
                prev = (b, sc, tb, e)
            av_for(prev)
            attTt = emit_finish_part1(*prev_sc, cur_att)
            finish_q.extend(finish_part2_quanta(*prev_sc, attTt))
            while pending or finish_q:
                pump(10000, NSC)

    nc.compile()
    return nc


def _prep_inputs(x, Wq, Wk, Wv, Wo, Wlq, blq, Wlk, blk):
    bf = ml_dtypes.bfloat16
    f8 = ml_dtypes.float8_e4m3
    x = np.asarray(x, np.float64)
    xT = np.ascontiguousarray(x.reshape(BS, D).T).reshape(KB, 128, BS)
    xh = xT.astype(f8)
    xhf = xh.astype(np.float64)
    xl = ((xT - xhf) * 8).astype(f8)
    xs = (xhf / 16).astype(f8)

    Wq = np.asarray(Wq, np.float64)
    Wk = np.asarray(Wk, np.float64)
    Wv = np.asarray(Wv, np.float64)
    Wo = np.asarray(Wo, np.float64)
    Wlq = np.asarray(Wlq, np.float64)
    Wlk = np.asarray(Wlk, np.float64)
    blq64 = np.asarray(blq, np.float64)
    blk64 = np.asarray(blk, np.float64)

    def sbl(w_c):  # [128 rows, D] -> [128, D] kb-major lhsT layout
        return np.ascontiguousarray(
            w_c.T.reshape(KB, 128, 128).transpose(1, 0, 2).reshape(128, D))

    wf = np.zeros((128, 4), np.float32)
    wf[0:HD, 0] = blq64
    wf[HD:128, 0] = blq64
    wf[0:HD, 1] = blk64
    wf[HD:128, 1] = blk64
    idb = np.eye(128).astype(bf)

    def split3(w_f):
        # [128, D] fused weight -> [128, 3*D] fp8 terms (h16, h2, l256)
        s = sbl(w_f)
        h16 = (s * 16).astype(f8)
        h16f = h16.astype(np.float64)
        h2 = (h16f / 8).astype(f8)
        l256 = ((s * 16 - h16f) * 16).astype(f8)
        return np.concatenate([h16, h2, l256], axis=1)

    in_maps = []
    for c in range(NC):
        r = slice(c * 128, (c + 1) * 128)
        wq_f = np.empty((128, D), np.float64)
        wk_f = np.empty((128, D), np.float64)
        wq_c, wk_c = Wq[r, :], Wk[r, :]
        wq_f[0:HD] = Wlq @ wq_c[0:HD]
        wq_f[HD:128] = Wlq @ wq_c[HD:128]
        wk_f[0:HD] = Wlk @ wk_c[0:HD]
        wk_f[HD:128] = Wlk @ wk_c[HD:128]
        in_maps.append({
            "xh": xh, "xl": xl, "xs": xs,
            "wq": split3(wq_f),
            "wk": split3(wk_f),
            "wv": split3(Wv[r, :]),
            "wo": np.ascontiguousarray(Wo[:, r].T).astype(bf),
            "wf": wf,
            "idb": idb,
        })
    return in_maps


def kernel(x, Wq, Wk, Wv, Wo, Wlq, blq, Wlk, blk):
    if "nc" not in _cache:
        _cache["nc"] = build_nc()
    nc = _cache["nc"]
    in_maps = _prep_inputs(x, Wq, Wk, Wv, Wo, Wlq, blq, Wlk, blk)
    res = run_bass_kernel_spmd(nc, in_maps, core_ids=list(range(NC)))
    acc = np.zeros((KB, 128, BS), np.float32)
    for c in range(NC):
        acc += res.results[c]["outT"].astype(np.float32)
    out = acc.reshape(D, BS).T.reshape(B, S, D).astype(np.float32)
    return out


# revision 21
# speedup vs baseline: 1.0841x; 1.0388x over previous
"""DeepSeek-style attention, tensor-parallel over 8 TRN2 NeuronCores.

Sharding: 16 heads / 8 cores = 2 heads per core. Each core computes its
2 heads' projections, attention, and a partial output projection; the
host sums the 8 partial outputs.

v2 design notes (cost model: matmul = out_free_cols cycles; fp8
DoubleRow = 0.5 cycles/col; ACT/DVE/Pool = free-size elems/cycle):
  - latent transforms are linear -> fused host-side into Wq/Wk
  - QKV projections bf16, full PE rate; V computed pre-transposed
    (x as stationary operand) so no PE transposes are needed
  - scores: lq/lk quantized fp8e4, DoubleRow matmul with a zero
    second k-tile (mega tile = [lq | lk | zeros]) -> 256 cycles per
    (head, tblock, 512 queries)
  - exp on ACT (exact, bf16 out); optional Schraudolph offload of some
    tiles to DVE/Pool (tensor_scalar -> int16 bits of bf16)
  - AV flipped: e[t,s128] is the stationary operand, rhs = vaug[t,65]
    per head ([v | 1]; the ones column accumulates the softmax
    denominator) -> attended^T[s, ch] with den per-partition
  - normalize: reciprocal_approx_fast + per-partition-scale multiply,
    then PE bf16 transpose back to [ch, s] for the output projection
  - output projection bf16; partial outputs written bf16, host sums
"""
import numpy as np
import ml_dtypes

import concourse.mybir as mybir
import concourse.tile as tile
from concourse import bacc
from concourse.bass_utils import run_bass_kernel_spmd

F32 = mybir.dt.float32
BF16 = mybir.dt.bfloat16
FP8 = mybir.dt.float8e4
I16 = mybir.dt.int16
EXP = mybir.ActivationFunctionType.Exp
MUL = mybir.AluOpType.mult
ADD = mybir.AluOpType.add
DR = mybir.MatmulPerfMode.DoubleRow

H, D, HD = 16, 1024, 64
B, S = 2, 2048
BS = B * S          # 4096
KB = D // 128       # 8 k-blocks
NC = 8              # cores
SC = 512            # s-chunk width
NSC = BS // SC      # 8 chunks
TPC = SC // 128     # 4 t-blocks per chunk
NTB = S // 128      # 16 t-blocks per batch
VW = 2 * (HD + 1)   # 130 vaug cols per t-block

# exp engine per (b, sc, tb): 'A' = ACT exact, 'V' = DVE Schraudolph,
# 'P' = Pool Schraudolph.  Tuned against measured rel-err headroom.
EXP_ASSIGN = {}
SCHR_A = 0.125 * 128 * 1.4426950408889634   # scale folded in
SCHR_B = 128 * 127.0 + 0.5 - 5.0

_cache = {}
DEBUG = False


def exp_engine(b, sc, tb):
    return EXP_ASSIGN.get((b, sc, tb), "A")


def build_nc():
    nc = bacc.Bacc("TRN2", target_bir_lowering=False, debug=False)
    xh_d = nc.dram_tensor("xh", [KB, 128, BS], FP8, kind="ExternalInput").ap()
    xl_d = nc.dram_tensor("xl", [KB, 128, BS], FP8, kind="ExternalInput").ap()
    xs_d = nc.dram_tensor("xs", [KB, 128, BS], FP8, kind="ExternalInput").ap()
    wq_d = nc.dram_tensor("wq", [128, 3 * D], FP8, kind="ExternalInput").ap()
    wk_d = nc.dram_tensor("wk", [128, 3 * D], FP8, kind="ExternalInput").ap()
    wv_d = nc.dram_tensor("wv", [128, 3 * D], FP8, kind="ExternalInput").ap()
    wo_d = nc.dram_tensor("wo", [128, D], BF16, kind="ExternalInput").ap()
    wf_d = nc.dram_tensor("wf", [128, 4], F32, kind="ExternalInput").ap()
    idb_d = nc.dram_tensor("idb", [128, 128], BF16, kind="ExternalInput").ap()
    out_d = nc.dram_tensor("outT", [KB, 128, BS], BF16, kind="ExternalOutput").ap()
    if DEBUG:
        dbg_mega = nc.dram_tensor("dbg_mega", [128, 3 * BS], FP8, kind="ExternalOutput").ap()
        dbg_vaug = nc.dram_tensor("dbg_vaug", [128, 32 * VW], BF16, kind="ExternalOutput").ap()
        dbg_e = nc.dram_tensor("dbg_e", [128, 2 * SC], BF16, kind="ExternalOutput").ap()
        dbg_att = nc.dram_tensor("dbg_att", [128, 2 * VW], F32, kind="ExternalOutput").ap()
        dbg_attT = nc.dram_tensor("dbg_attT", [128, SC], BF16, kind="ExternalOutput").ap()

    with tile.TileContext(nc) as tc:
        with (
            tc.tile_pool(name="wpool", bufs=1) as wpool,
            tc.tile_pool(name="big", bufs=1) as big,
            tc.tile_pool(name="ep", bufs=4) as epool,
            tc.tile_pool(name="att2", bufs=2) as att2p,
            tc.tile_pool(name="st", bufs=2) as stpool,
            tc.tile_pool(name="scp", bufs=2, space="PSUM") as scpp,
            tc.tile_pool(name="attp", bufs=1, space="PSUM") as attpp,
            tc.tile_pool(name="pp", bufs=2, space="PSUM") as ppp,
        ):
            wq_t = wpool.tile([128, 3 * D], FP8, tag="wq")
            wk_t = wpool.tile([128, 3 * D], FP8, tag="wk")
            wv_t = wpool.tile([128, 3 * D], FP8, tag="wv")
            wo_t = wpool.tile([128, D], BF16, tag="wo")
            wf_t = wpool.tile([128, 4], F32, tag="wf")
            idb_t = wpool.tile([128, 128], BF16, tag="idb")
            nc.sync.dma_start(out=wq_t[:], in_=wq_d)
            # per-weight views [128, term, kb, 128]
            wq4 = wq_t[:].rearrange("p (t k n) -> p t k n", t=3, k=KB)
            wk4 = wk_t[:].rearrange("p (t k n) -> p t k n", t=3, k=KB)
            wv4 = wv_t[:].rearrange("p (t k n) -> p t k n", t=3, k=KB)
            blq_s = wf_t[:, 0:1]
            blk_s = wf_t[:, 1:2]

            # mega = [lq | lk | zeros], fp8, k-tile stride BS
            mega = big.tile([128, 3 * BS], FP8, tag="mega")
            mega3 = mega[:].rearrange("p (t n) -> p t n", t=3)
            nc.gpsimd.memset(mega3[:, 2], 0.0)

            vaug = big.tile([128, 32 * VW], BF16, tag="vaug")
            vaug3 = vaug[:].rearrange("p (t c) -> p t c", c=VW)
            vaug4 = vaug[:].rearrange("p (t h c) -> p t h c", h=2, c=HD + 1)
            nc.gpsimd.memset(vaug4[:, :, :, HD:HD + 1], 1.0)

            xh_t = big.tile([128, NSC * KB * SC], FP8, tag="xh")
            xl_t = big.tile([128, NSC * KB * SC], FP8, tag="xl")
            xs_t = big.tile([128, NSC * KB * SC], FP8, tag="xs")
            xh4 = xh_t[:].rearrange("p (c k n) -> p c k n", c=NSC, k=KB)
            xl4 = xl_t[:].rearrange("p (c k n) -> p c k n", c=NSC, k=KB)
            xs4 = xs_t[:].rearrange("p (c k n) -> p c k n", c=NSC, k=KB)
            for t4, d in ((xh4, xh_d), (xl4, xl_d), (xs4, xs_d)):
                nc.sync.dma_start(
                    out=t4[:, 0],
                    in_=d[:, :, 0:SC].rearrange("k p n -> p k n"))
            for t, d in ((wk_t, wk_d), (wv_t, wv_d), (wo_t, wo_d),
                         (wf_t, wf_d), (idb_t, idb_d)):
                nc.sync.dma_start(out=t[:], in_=d)
            for c in range(1, NSC):
                for t4, d in ((xh4, xh_d), (xl4, xl_d), (xs4, xs_d)):
                    nc.sync.dma_start(
                        out=t4[:, c],
                        in_=d[:, :, c * SC:(c + 1) * SC].rearrange("k p n -> p k n"),
                    )

            # warm the PE p-state during the initial DMA: ~4us of junk
            # matmuls on already-loaded weights (results never read)
            warm = ppp.tile([128, SC], F32, tag="pp")
            for i in range(5):
                nc.tensor.matmul(
                    warm[:], wq_t[:, 0:128], wq_t[:, 0:SC],
                    start=(i == 0), stop=(i == 4))
            X3 = (xh4, xl4, xs4)  # matched term scales: 16*(x@W)

            # ---------------- emission quanta ----------------
            def chunk_groups(c):
                """Phase 1 for token chunk c as groups of quanta.  Quanta
                within a group share a ppp psum tile and must not interleave
                with other ppp users (the pump enforces this)."""
                col = c * SC
                state = {}

                def proj_piece(key, w4, term, kb0, bias, dst):
                    # DoubleRow over kb pairs; 3 scale-matched fp8 terms
                    # accumulate 16*(x@W) into one psum
                    def f():
                        if term == 0 and kb0 == 0:
                            pt = ppp.tile([128, SC], F32, tag="pp")
                            state[key] = pt
                        p = state[key]
                        xv = X3[term][:, c]
                        for kb in range(kb0, kb0 + 4, 2):
                            nc.tensor.matmul(
                                p[:],
                                w4[:, term, kb:kb + 2, :],
                                xv[:, kb:kb + 2, :],
                                start=(term == 0 and kb == 0),
                                stop=(term == 2 and kb == KB - 2),
                                perf_mode=DR)
                        if term == 2 and kb0 + 4 == KB:
                            nc.vector.tensor_scalar(
                                out=dst, in0=p[:], scalar1=0.0625,
                                scalar2=bias, op0=MUL, op1=ADD)
                    return f

                def proj_group(key, w4, bias, dst):
                    return [proj_piece(key, w4, term, kb0, bias, dst)
                            for term in range(3) for kb0 in (0, 4)]

                def q_v(i):
                    def f():
                        vp = ppp.tile([128, 128], F32, tag="pp")
                        for term in range(3):
                            xv = X3[term][:, c]
                            for kb in range(0, KB, 2):
                                nc.tensor.matmul(
                                    vp[:],
                                    xv[:, kb:kb + 2, i * 128:(i + 1) * 128],
                                    wv4[:, term, kb:kb + 2, :],
                                    start=(term == 0 and kb == 0),
                                    stop=(term == 2 and kb == KB - 2),
                                    perf_mode=DR)
                        tbg = c * TPC + i
                        nc.vector.tensor_scalar_mul(
                            vaug4[:, tbg, :, 0:HD],
                            vp[:].rearrange("p (h c) -> p h c", h=2),
                            0.0625)
                    return f

                return [
                    [(q, 240) for q in proj_group("lq", wq4, blq_s,
                                                  mega3[:, 0, col:col + SC])],
                    [(q, 240) for q in proj_group("lk", wk4, blk_s,
                                                  mega3[:, 1, col:col + SC])],
                    [(q_v(0), 350)], [(q_v(1), 350)],
                    [(q_v(2), 350)], [(q_v(3), 350)],
                ]

            def emit_scores(b, sc, tb):
                """Scores (fp8 DoubleRow) + exp for one t-block; returns e."""
                scol = b * S + sc * SC
                tcol = b * S + tb * 128
                scp = scpp.tile([128, 2 * SC], F32, tag="scp")
                for h in range(2):
                    nc.tensor.matmul(
                        scp[:, h * SC:(h + 1) * SC],
                        mega3[64 * h:64 * h + 64, 1:3, tcol:tcol + 128],
                        mega3[64 * h:64 * h + 64, 0:2, scol:scol + SC],
                        start=True, stop=True, perf_mode=DR,
                        tile_position=(64 * h, 0))
                e = epool.tile([128, 2 * SC], BF16, tag="e")
                eng = exp_engine(b, sc, tb)
                if eng == "A":
                    nc.scalar.activation(e[:], scp[:], EXP, scale=0.125)
                else:
                    veng = nc.vector if eng == "V" else nc.gpsimd
                    veng.tensor_scalar(
                        out=e[:].bitcast(I16), in0=scp[:],
                        scalar1=SCHR_A, scalar2=SCHR_B, op0=MUL, op1=ADD)
                if DEBUG and b == 0 and sc == 0 and tb == 0:
                    nc.sync.dma_start(out=dbg_e, in_=e[:])
                return e

            def emit_av(b, sc, tb, e, att_ts):
                tbg = b * NTB + tb
                for q in range(4):
                    att = att_ts[q // 2]
                    for h in range(2):
                        nc.tensor.matmul(
                            att[:, q % 2, h * (HD + 1):(h + 1) * (HD + 1)],
                            e[:, h * SC + q * 128:h * SC + (q + 1) * 128],
                            vaug3[:, tbg, h * (HD + 1):(h + 1) * (HD + 1)],
                            start=False, stop=(tb == NTB - 1),
                            skip_group_check=True)

            def emit_finish_part1(b, sc, att_ts):
                """Normalize: recip + per-partition scale into attTt.
                Emitted immediately after AV(sc, 15) so the att psum tiles
                free up for the next s-chunk."""
                if DEBUG and b == 0 and sc == 0:
                    dbg_att_s = att2p.tile([128, 2 * VW], F32, tag="dbga")
                    nc.vector.tensor_copy(
                        out=dbg_att_s[:].rearrange("p (s c) -> p s c", c=VW),
                        in_=att_ts[0][:])
                    nc.sync.dma_start(out=dbg_att, in_=dbg_att_s[:])
                rec = att2p.tile([128, 8], F32, tag="rec")
                attTt = att2p.tile([128, 4 * 128], BF16, tag="attTt")
                for t_i in range(2):
                    a4 = att_ts[t_i][:].rearrange("p s (h c) -> p s h c", c=HD + 1)
                    nc.vector.reciprocal_approx_fast(
                        out=rec[:, 4 * t_i:4 * t_i + 4].rearrange(
                            "p (s h) -> p s h", s=2),
                        in_=a4[:, :, :, HD:HD + 1].rearrange("p s h o -> p s (h o)"))
                for q in range(4):
                    att = att_ts[q // 2]
                    a3 = att[:, q % 2].rearrange("p (h c) -> p h c", c=HD + 1)
                    last = (b == B - 1 and sc == NSC // B - 1)
                    for h in range(2):
                        dst = attTt[:, q * 128 + h * HD:q * 128 + (h + 1) * HD]
                        if last:
                            nc.scalar.mul(dst, a3[:, h, 0:HD],
                                          rec[:, 2 * q + h:2 * q + h + 1])
                        else:
                            nc.vector.tensor_scalar_mul(
                                dst, a3[:, h, 0:HD],
                                rec[:, 2 * q + h:2 * q + h + 1])
                return attTt

            def finish_part2_quanta(b, sc, attTt):
                """Transpose attended back to [ch, s], out-proj, stage, DMA."""
                scol = b * S + sc * SC
                attT = att2p.tile([128, SC], BF16, tag="attT")

                def q_tr(qr):
                    def f():
                        for q in qr:
                            tp = ppp.tile([128, 128], BF16, tag="pp")
                            nc.tensor.transpose(
                                tp[:], attTt[:, q * 128:(q + 1) * 128], idb_t[:])
                            nc.vector.tensor_copy(
                                out=attT[:, q * 128:(q + 1) * 128], in_=tp[:])
                        if DEBUG and b == 0 and sc == 0 and qr[-1] == 3:
                            nc.sync.dma_start(out=dbg_attT, in_=attT[:])
                    return f

                def q_oproj(pair):
                    def f():
                        stage = stpool.tile([128, 2 * SC], BF16, tag="stage")
                        for jj in range(2):
                            j = pair * 2 + jj
                            pop = ppp.tile([128, SC], F32, tag="pp")
                            nc.tensor.matmul(
                                pop[:], wo_t[:, j * 128:(j + 1) * 128], attT[:],
                                start=True, stop=True)
                            if b == B - 1 and sc == NSC // B - 1 and jj == 0:
                                nc.scalar.copy(
                                    out=stage[:, jj * SC:(jj + 1) * SC], in_=pop[:])
                            else:
                                nc.vector.tensor_copy(
                                    out=stage[:, jj * SC:(jj + 1) * SC], in_=pop[:])
                        nc.sync.dma_start(
                            out=out_d[pair * 2:pair * 2 + 2, :, scol:scol + SC]
                                .rearrange("k p n -> p k n"),
                            in_=stage[:].rearrange("p (k n) -> p k n", k=2),
                        )
                    return f

                return [(q_tr((0, 1)), 110), (q_tr((2, 3)), 110),
                        (q_oproj(0), 430), (q_oproj(1), 430),
                        (q_oproj(2), 430), (q_oproj(3), 430)]

            # ---------------- software-pipelined emission ----------------
            # Slot order is [scores(i), pump, AV(i-1)]: pumped phase-1 /
            # finish work runs while exp(i-1) is in flight on ACT, and the
            # next scores follow the AV drain immediately.
            pending = []        # (chunk_idx, quantum, last_in_group)
            finish_q = []
            in_group = [False]

            def add_chunk(c, groups):
                for g in groups:
                    for i, (q, cost) in enumerate(g):
                        pending.append((c, q, i == len(g) - 1, cost))

            g0 = chunk_groups(0)
            for q, _ in g0[0] + g0[1]:  # chunk0 lq + lk: critical path
                q()
            add_chunk(0, g0[2:])
            for c in range(1, NSC):
                add_chunk(c, chunk_groups(c))

            def pump(budget, due):
                # Drain quanta up to ~budget ns of PE work; only start chunk
                # work that is due within the next 2 chunks.
                while budget > 0:
                    if in_group[0] and pending:
                        _, q, last, cost = pending.pop(0)
                        q()
                        in_group[0] = not last
                        budget -= cost
                    elif finish_q and finish_q[0][1] <= budget:
                        q, cost = finish_q.pop(0)
                        q()
                        budget -= cost
                    elif (pending and pending[0][0] <= due + 3
                          and pending[0][3] <= budget):
                        _, q, last, cost = pending.pop(0)
                        q()
                        in_group[0] = not last
                        budget -= cost
                    else:
                        break

            def pump_until_chunk(cidx):
                while pending and (in_group[0] or pending[0][0] <= cidx):
                    _, q, last, cost = pending.pop(0)
                    q()
                    in_group[0] = not last

            slots = [(b, sc, tb)
                     for b in range(B)
                     for sc in range(NSC // B)
                     for tb in range(NTB)]
            prev = None          # (b, sc, tb, e)
            cur_att = None
            prev_sc = None

            def av_for(slot_state):
                nonlocal cur_att, prev_sc
                b, sc, tb, e = slot_state
                if tb == 0:
                    if cur_att is not None:
                        attTt = emit_finish_part1(*prev_sc, cur_att)
                        finish_q.extend(finish_part2_quanta(*prev_sc, attTt))
                    att_a = attpp.tile([128, 2, VW], F32, tag="att_a")
                    att_b = attpp.tile([128, 2, VW], F32, tag="att_b")
                    cur_att = (att_a, att_b)
                    prev_sc = (b, sc)
                    nc.vector.memset(att_a[:], 0.0)
                    nc.vector.memset(att_b[:], 0.0)
                emit_av(b, sc, tb, e, cur_att)

            for b, sc, tb in slots:
                due = b * (NSC // B) + max(sc, tb // TPC)
                pump_until_chunk(due)
                e = emit_scores(b, sc, tb)
                if prev is not None:
                    av_for(prev)
                prev = (b, sc, tb, e)
                pump(550, due)
            av_for(prev)
            attTt = emit_finish_part1(*prev_sc, cur_att)
            finish_q.extend(finish_part2_quanta(*prev_sc, attTt))
            while pending or finish_q:
                pump(10000, NSC)

    nc.compile()
    return nc


def _prep_inputs(x, Wq, Wk, Wv, Wo, Wlq, blq, Wlk, blk):
    bf = ml_dtypes.bfloat16
    f8 = ml_dtypes.float8_e4m3
    x = np.asarray(x, np.float64)
    xT = np.ascontiguousarray(x.reshape(BS, D).T).reshape(KB, 128, BS)
    xh = xT.astype(f8)
    xhf = xh.astype(np.float64)
    xl = ((xT - xhf) * 8).astype(f8)
    xs = (xhf / 16).astype(f8)

    Wq = np.asarray(Wq, np.float64)
    Wk = np.asarray(Wk, np.float64)
    Wv = np.asarray(Wv, np.float64)
    Wo = np.asarray(Wo, np.float64)
    Wlq = np.asarray(Wlq, np.float64)
    Wlk = np.asarray(Wlk, np.float64)
    blq64 = np.asarray(blq, np.float64)
    blk64 = np.asarray(blk, np.float64)

    def sbl(w_c):  # [128 rows, D] -> [128, D] kb-major lhsT layout
        return np.ascontiguousarray(
            w_c.T.reshape(KB, 128, 128).transpose(1, 0, 2).reshape(128, D))

    wf = np.zeros((128, 4), np.float32)
    wf[0:HD, 0] = blq64
    wf[HD:128, 0] = blq64
    wf[0:HD, 1] = blk64
    wf[HD:128, 1] = blk64
    idb = np.eye(128).astype(bf)

    def split3(w_f):
        # [128, D] fused weight -> [128, 3*D] fp8 terms (h16, h2, l256)
        s = sbl(w_f)
        h16 = (s * 16).astype(f8)
        h16f = h16.astype(np.float64)
        h2 = (h16f / 8).astype(f8)
        l256 = ((s * 16 - h16f) * 16).astype(f8)
        return np.concatenate([h16, h2, l256], axis=1)

    in_maps = []
    for c in range(NC):
        r = slice(c * 128, (c + 1) * 128)
        wq_f = np.empty((128, D), np.float64)
        wk_f = np.empty((128, D), np.float64)
        wq_c, wk_c = Wq[r, :], Wk[r, :]
        wq_f[0:HD] = Wlq @ wq_c[0:HD]
        wq_f[HD:128] = Wlq @ wq_c[HD:128]
        wk_f[0:HD] = Wlk @ wk_c[0:HD]
        wk_f[HD:128] = Wlk @ wk_c[HD:128]
        in_maps.append({
            "xh": xh, "xl": xl, "xs": xs,
            "wq": split3(wq_f),
            "wk": split3(wk_f),
            "wv": split3(Wv[r, :]),
            "wo": np.ascontiguousarray(Wo[:, r].T).astype(bf),
            "wf": wf,
            "idb": idb,
        })
    return in_maps


def kernel(x, Wq, Wk, Wv, Wo, Wlq, blq, Wlk, blk):
    if "nc" not in _cache:
        _cache["nc"] = build_nc()
    nc = _cache["nc"]
    in_maps = _prep_inputs(x, Wq, Wk, Wv, Wo, Wlq, blq, Wlk, blk)
    res = run_bass_kernel_spmd(nc, in_maps, core_ids=list(range(NC)))
    acc = np.zeros((KB, 128, BS), np.float32)
    for c in range(NC):
        acc += res.results[c]["outT"].astype(np.float32)
    out = acc.reshape(D, BS).T.reshape(B, S, D).astype(np.float32)
    return out
